# revision 2
# baseline (speedup 1.0000x reference)
"""Two-layer GAT on 8 Trainium2 NeuronCores (Bass/Tile), bf16 gather path.

Host (numpy): append self-loops, degree-sort nodes (desc), pad node count to
VPAD (multiple of 8*128) and assign sorted nodes round-robin at 128-node
block granularity to the 8 cores (sorted-rank s -> block g=s//128,
lane=s%128 -> core c=g%8, local block j=g//8, table row = c*PC+j*128+lane).
Per block-rank j the chunk schedule is shared by all cores (SPMD: one
program, per-core tensor data).  Each dst node's edges occupy "slots"; a
chunk is slot k of all 128 lanes of a block.  Edge slots are split across
WINS_N overlapping 32768-row windows of the table (dma_gather indices are
int16; a left-to-right greedy balances each dst's edges over the windows
it is eligible for, minimizing the shared max-over-lanes chunk counts --
~1.25x padding vs 1.75x for a fixed half split).  Gather calls rotate
over the 4 SWDGE queues (4 concurrent Q7 descriptor generators, ~3x the
single-queue rate); per-call tiles are sized to the largest real call so
the gather pipeline can run 6 calls deep.

The gather table holds ONLY h (+bias) as 128 bf16 = 256B rows (the dma
gather minimum).  Per-edge alpha_src is recomputed on-chip from the
gathered h row via a DVE multiply + free-dim reduce against a replicated
a_src constant; alpha_dst stays a per-lane [128,1] activation bias
(lane-aligned slots).  Softmax denominators come from a tensor_reduce over
the per-block z tile instead of extra V columns, so the segment-sum matmul
V is a pure 128-col bf16 tile accumulated into PSUM with an identity lhsT.
x, W, the table, and V are all bf16; PSUM accumulation fp32.
"""

import numpy as np
import ml_dtypes

BF16 = ml_dtypes.bfloat16

NCORES = 8
F_IN = 128
HID = 64
HEADS = 2
OUT = 64
NEG_SLOPE = 0.2

GBATCH = 64  # max chunks per dma_gather
KMAX = 64    # z-tile slot capacity (assert K.max() <= KMAX)
PIECES = 2   # AllGather pieces (table row space is piece-major)
WINS_N = 3   # int16 gather windows over the table
GROUP = 1    # blocks whose window chunks share gather calls (= psum tiles
             # concurrently accumulating; bounded by PSUM banks)

TRACE = False
_cache = {}


def _build_schedule(edge_index, n_nodes):
    ei = np.asarray(edge_index).astype(np.int64)
    src = np.concatenate([ei[0], np.arange(n_nodes, dtype=np.int64)])
    dst = np.concatenate([ei[1], np.arange(n_nodes, dtype=np.int64)])
    deg = np.bincount(dst, minlength=n_nodes)

    stripe = NCORES * 128
    vpad = ((n_nodes + stripe - 1) // stripe) * stripe
    pc = vpad // NCORES
    nb = pc // 128
    assert vpad <= 3 * 32768

    # AllGather piece boundaries (in blocks); the gather-table row space is
    # piece-major (piece, core, block-within-piece, lane) so each piece's
    # collective output is one contiguous slice of h_full.
    cuts = sorted(set(min(nb, ((nb * (i + 1)) + PIECES - 1) // PIECES)
                      for i in range(PIECES)))
    starts = [0] + cuts[:-1]
    piece_of_j = np.zeros(nb, np.int64)
    tbase_of_j = np.zeros(nb, np.int64)
    cstr_of_j = np.zeros(nb, np.int64)
    joff_of_j = np.zeros(nb, np.int64)
    base = 0
    for p, (j0, j1) in enumerate(zip(starts, cuts)):
        bp = j1 - j0
        for j in range(j0, j1):
            piece_of_j[j] = p
            tbase_of_j[j] = base
            cstr_of_j[j] = bp * 128
            joff_of_j[j] = (j - j0) * 128
        base += NCORES * bp * 128
    assert base == vpad

    degp = np.zeros(vpad, np.int64)
    degp[:n_nodes] = deg
    order = np.argsort(-degp, kind="stable")
    rank = np.empty(vpad, np.int64)
    rank[order] = np.arange(vpad)

    s = np.arange(vpad)
    g = s // 128
    lane_r = s % 128
    c_r = g % NCORES
    j_r = g // NCORES
    local_of_rank = c_r * pc + j_r * 128 + lane_r
    trow_of_rank = (tbase_of_j[j_r] + c_r * cstr_of_j[j_r]
                    + joff_of_j[j_r] + lane_r)
    nrank = rank[:n_nodes]
    row_of_node = local_of_rank[nrank]       # for x placement / output
    trow_of_node = trow_of_rank[nrank]       # for gather indices
    c_of_node = c_r[nrank]
    j_of_node = j_r[nrank]
    lane_of_node = lane_r[nrank]

    e_srcrow = trow_of_node[src]
    e_c = c_of_node[dst]
    e_j = j_of_node[dst]
    e_lane = lane_of_node[dst]
    e_dkey = (e_c * nb + e_j) * 128 + e_lane  # dense (c, j, lane) id
    nkeys = NCORES * nb * 128

    # int16-addressable gather windows over the table rows; per-dst edges are
    # split across windows (balanced via the overlap regions) to minimize the
    # shared max-over-lanes chunk counts.  Left-to-right greedy: window w must
    # take every not-yet-assigned row below the next window's start, and takes
    # up to the block max of that forced count from its eligible range.
    W = 32768
    if vpad <= W:
        wins = [(0, vpad)]
    else:
        starts = [round((vpad - W) * i / (WINS_N - 1)) for i in range(WINS_N)]
        wins = [(st, st + W) for st in starts]
    NW = len(wins)

    def cnt(mask):
        return np.bincount(e_dkey[mask], minlength=nkeys)

    dcnt = cnt(np.ones(e_srcrow.shape[0], bool))
    jj = (np.arange(nkeys) // 128) % nb

    def blockmax(x):
        m = np.zeros(nb, np.int64)
        np.maximum.at(m, jj, x)
        return m

    assigned = np.zeros(nkeys, np.int64)
    Kw = []
    take_w = []
    for w in range(NW):
        nxt = wins[w + 1][0] if w + 1 < NW else vpad
        need = np.maximum(cnt(e_srcrow < nxt) - assigned, 0)
        K_w = blockmax(need)
        take = np.minimum(K_w[jj], cnt(e_srcrow < wins[w][1]) - assigned)
        take = np.maximum(take, need)
        assigned += take
        Kw.append(K_w)
        take_w.append(take)
    assert (assigned == dcnt).all()

    K = np.sum(Kw, axis=0)
    bump = K == 0
    Kw[0][bump] += 1
    K[bump] += 1

    # group-major chunk layout: per GROUP of blocks, per window, per block
    groups = [(j0, min(j0 + GROUP, nb)) for j0 in range(0, nb, GROUP)]
    cb = np.zeros((NW, nb), np.int64)
    cursor = 0
    for (j0, j1) in groups:
        for w in range(NW):
            for j in range(j0, j1):
                cb[w, j] = cursor
                cursor += int(Kw[w][j])
    nch = cursor
    assert nch == int(K.sum())

    # per-edge window: sort edges by (dst, src); window w takes the next
    # take_w[w] edges of each dst in src order
    ord_e = np.lexsort((e_srcrow, e_dkey))
    ds = e_dkey[ord_e]
    sr = e_srcrow[ord_e]
    first = np.r_[True, ds[1:] != ds[:-1]]
    grp_start = np.flatnonzero(first)
    grp_id = np.cumsum(first) - 1
    rank_in = np.arange(ds.shape[0]) - grp_start[grp_id]
    cumtake = np.cumsum(np.stack(take_w), axis=0)  # [NW, nkeys]
    ewin = (rank_in[:, None] >= cumtake[:, ds].T).sum(1)
    prev = np.where(ewin > 0,
                    cumtake[np.maximum(ewin - 1, 0), ds], 0)
    eslot = rank_in - prev
    j = (ds // 128) % nb
    ln = ds % 128
    c = ds // (nb * 128)
    pos = cb[ewin, j] + eslot
    assert (eslot >= 0).all()
    assert (eslot < np.stack([k[j] for k in Kw])[ewin,
                                                 np.arange(len(j))]).all()
    wlo = np.array([wins[w][0] for w in range(NW)])
    # every edge's src row must lie inside its window
    assert (sr >= wlo[ewin]).all()
    assert (sr < np.array([wins[w][1] for w in range(NW)])[ewin]).all()

    idx_val = sr - wlo[ewin]
    idx_stream = np.zeros((NCORES, 128, nch), np.int16)
    mask_stream = np.zeros((NCORES, 128, nch), BF16)
    idx_stream[c, ln, pos] = idx_val.astype(np.int16)
    mask_stream[c, ln, pos] = 1.0

    # wrapped int16 layout for dma_gather: chunk k -> columns 8k:8k+8 of
    # [128, 8*nch]; within a chunk the 128 lane-indices are wrapped as
    # flat[i] -> [i % 16, i // 16] and replicated over the 8 16-partition
    # groups.
    iw = idx_stream.transpose(0, 2, 1).reshape(NCORES, nch, 8, 16)
    iw = iw.transpose(0, 3, 1, 2).reshape(NCORES, 16, nch * 8)
    idx_wrapped = np.tile(iw, (1, 8, 1))

    return dict(vpad=vpad, pc=pc, nb=nb, wins=tuple(wins),
                Kw=tuple(tuple(k.tolist()) for k in Kw), K=K,
                nch=nch, cb=tuple(tuple(r.tolist()) for r in cb),
                groups=tuple(groups), row_of_node=row_of_node,
                cuts=tuple(cuts),
                idx_wrapped=np.ascontiguousarray(idx_wrapped),
                mask_stream=np.ascontiguousarray(mask_stream))


def _build_program(vpad, pc, nb, wins, Kw, K, nch, cb, groups, cuts):
    import concourse.bacc as bacc
    import concourse.bass as bass
    import concourse.mybir as mybir
    import concourse.tile as tile
    from concourse.masks import make_identity

    F32 = mybir.dt.float32
    BF = mybir.dt.bfloat16
    I16 = mybir.dt.int16
    ACTF = mybir.ActivationFunctionType
    ALU = mybir.AluOpType
    AXL = mybir.AxisListType

    NW = len(wins)
    firstq = {}
    lastq = {}
    for j in range(nb):
        ws = [w for w in range(NW) if Kw[w][j] > 0]
        firstq[j] = min(cb[w][j] for w in ws)
        lastq[j] = max(cb[w][j] + Kw[w][j] - 1 for w in ws)
    # largest single gather call (chunks): group batches are split at GBATCH
    gb = min(GBATCH, max(max(k) for k in Kw) * GROUP)

    nc = bacc.Bacc("TRN2", target_bir_lowering=False, debug=False,
                   num_devices=NCORES, num_swdge_queues=4)
    qn = [0]

    def next_q():
        q = qn[0]
        qn[0] = (q + 1) % 4
        return q

    xt_d = nc.dram_tensor("xt", [128, pc], BF, kind="ExternalInput")
    idx_d = nc.dram_tensor("idx", [128, nch * 8], I16, kind="ExternalInput")
    msk_d = nc.dram_tensor("msk", [128, nch], BF, kind="ExternalInput")
    w1_d = nc.dram_tensor("w1aug", [128, 130], BF, kind="ExternalInput")
    w2_d = nc.dram_tensor("w2aug", [128, 66], BF, kind="ExternalInput")
    b1_d = nc.dram_tensor("b1rep", [128, 128], F32, kind="ExternalInput")
    b2_d = nc.dram_tensor("b2rep", [128, 64], F32, kind="ExternalInput")
    as_d = nc.dram_tensor("asrcrep", [128, 128], BF, kind="ExternalInput")
    out_d = nc.dram_tensor("out", [pc, OUT], F32, kind="ExternalOutput")

    with tile.TileContext(nc) as tc:
        with (
            tc.tile_pool(name="const", bufs=1) as cp,
            tc.tile_pool(name="dram", bufs=1, space="DRAM") as dp,
            tc.tile_pool(name="hrow", bufs=3) as hp,
            tc.tile_pool(name="psh", bufs=2, space="PSUM") as psh,
            tc.tile_pool(name="g", bufs=6) as gp,
            tc.tile_pool(name="sc", bufs=2) as sp,
            tc.tile_pool(name="v", bufs=3) as vp,
            tc.tile_pool(name="z", bufs=3) as zp,
            tc.tile_pool(name="wz", bufs=4) as wp,
            tc.tile_pool(name="psa", bufs=4, space="PSUM") as psa,
            tc.tile_pool(name="pst", bufs=1, space="PSUM") as pst,
            tc.tile_pool(name="epi", bufs=3) as ep,
        ):
            ident = cp.tile([128, 128], BF)
            make_identity(nc, ident[:])
            w1_sb = cp.tile([128, 130], BF)
            w2_sb = cp.tile([128, 66], BF)
            b1_sb = cp.tile([128, 128], F32)
            b2_sb = cp.tile([128, 64], F32)
            as_sb = cp.tile([128, 128], BF)
            idx_t = cp.tile([128, nch * 8], I16)
            msk_t = cp.tile([128, nch], BF)
            xt_all = cp.tile([128, pc], BF)
            adst1 = cp.tile([128, 2 * nb], F32)
            adst2 = cp.tile([128, nb], F32)
            for t, d in ((w1_sb, w1_d), (w2_sb, w2_d), (b1_sb, b1_d),
                         (b2_sb, b2_d), (as_sb, as_d), (idx_t, idx_d),
                         (msk_t, msk_d), (xt_all, xt_d)):
                nc.sync.dma_start(out=t[:], in_=d[:])

            h_loc = dp.tile([pc, 128], BF)
            h2_loc = dp.tile([pc, 128], BF)
            h_full = dp.tile([vpad, 128], BF)
            h2_full = dp.tile([vpad, 128], BF)

            # piece p = blocks [starts[p], cuts[p]); its table rows are the
            # contiguous [tb, tb + 8*bp*128) slice (piece-major row space)
            starts = [0] + list(cuts[:-1])
            tbases = []
            tb = 0
            for j0, j1 in zip(starts, cuts):
                tbases.append(tb)
                tb += NCORES * (j1 - j0) * 128
            assert tb == vpad

            def ag_piece(loc, full, p):
                j0, j1 = starts[p], cuts[p]
                nc.gpsimd.collective_compute(
                    "AllGather", mybir.AluOpType.bypass,
                    replica_groups=[list(range(NCORES))],
                    ins=[loc[j0 * 128:j1 * 128, :]],
                    outs=[full[tbases[p]:tbases[p]
                               + NCORES * (j1 - j0) * 128, :]],
                )

            # ---- Phase 1: L1 h-phase (AllGather in overlapped pieces) ----
            for j in range(nb):
                ps = psh.tile([128, 130], F32, tag="psh")
                nc.tensor.matmul(ps[:], lhsT=xt_all[:, j * 128:(j + 1) * 128],
                                 rhs=w1_sb[:], start=True, stop=True)
                hrow = hp.tile([128, 128], BF, tag="hrow")
                nc.vector.tensor_tensor(out=hrow[:], in0=ps[:, 0:128],
                                        in1=b1_sb[:], op=ALU.add)
                nc.scalar.copy(adst1[:, j * 2:(j + 1) * 2], ps[:, 128:130])
                nc.sync.dma_start(out=h_loc[j * 128:(j + 1) * 128, :],
                                  in_=hrow[:])
                if j + 1 in cuts:
                    ag_piece(h_loc, h_full, cuts.index(j + 1))

            def group_batches(j0, j1, w):
                """Contiguous chunk batches (k0, gl, segs) of the group's
                window-w chunks; segs = [(j, call-offset, len), ...]."""
                segs = [(j, cb[w][j], Kw[w][j]) for j in range(j0, j1)
                        if Kw[w][j] > 0]
                if not segs:
                    return
                q0 = segs[0][1]
                qend = segs[-1][1] + segs[-1][2]
                b = q0
                while b < qend:
                    gl = min(gb, qend - b)
                    sub = []
                    for (j, s0, sl) in segs:
                        a = max(s0, b)
                        e = min(s0 + sl, b + gl)
                        if a < e:
                            sub.append((j, a - b, e - a))
                    yield b, gl, sub
                    b += gl

            def mm_rhs(psums, blk_of, k0, gl, vt, vcols):
                for k in range(gl):
                    q = k0 + k
                    j = blk_of[q]
                    nc.tensor.matmul(
                        psums[j][:], lhsT=ident[:],
                        rhs=vt[:, k * vcols:(k + 1) * vcols],
                        start=(q == firstq[j]), stop=(q == lastq[j]))

            blk_of = np.zeros(nch, np.int64)
            for j in range(nb):
                for w in range(NW):
                    blk_of[cb[w][j]:cb[w][j] + Kw[w][j]] = j

            def agg1_group(j0, j1):
                psums = {j: psa.tile([128, 130], F32, tag="ps",
                                     name=f"ps1_{j}")
                         for j in range(j0, j1)}
                for w in range(NW):
                    for (k0, gl, sub) in group_batches(j0, j1, w):
                        gt = gp.tile([128, gb * 128], BF, tag="g")
                        tab_ap = h_full[wins[w][0]:wins[w][1], :]
                        nc.gpsimd.dma_gather(
                            gt[:, 0:gl * 128].rearrange(
                                "p (k c) -> p k c", c=128),
                            tab_ap,
                            idx_t[:, k0 * 8:(k0 + gl) * 8],
                            gl * 128, gl * 128, 128,
                            single_packet=False, queue_num=next_q(),
                        )
                        gv = gt[:, 0:gl * 128].rearrange(
                            "p (k c) -> p k c", c=128)
                        az = zp.tile([128, 2, gb], F32, tag="az")
                        vt = vp.tile([128, gb * 130], BF, tag="v")
                        vv = vt[:, 0:gl * 130].rearrange(
                            "p (k c) -> p k c", c=130)
                        # alpha_src per edge: dot(h_row, a_src) per head,
                        # both heads in one mult + one reduce
                        scr = sp.tile([128, gb * 128], BF, tag="s")
                        sv = scr[:, 0:gl * 128].rearrange(
                            "p (k h c) -> p k h c", h=2, c=HID)
                        nc.vector.tensor_tensor(
                            out=sv,
                            in0=gv.rearrange("p k (h c) -> p k h c", c=HID),
                            in1=as_sb[:].rearrange("p (h c) -> p h c", c=HID)
                                .unsqueeze(1).broadcast_to([128, gl, 2, HID]),
                            op=ALU.mult)
                        azr = az[:, :, :].rearrange("p h k -> p k h")
                        nc.vector.tensor_reduce(
                            out=azr[:, 0:gl, :], in_=sv,
                            axis=AXL.X, op=ALU.add)
                        for (j, o0, ol) in sub:
                            for h in range(HEADS):
                                nc.scalar.activation(
                                    az[:, h, o0:o0 + ol], az[:, h, o0:o0 + ol],
                                    ACTF.Prelu,
                                    bias=adst1[:, 2 * j + h:2 * j + h + 1],
                                    alpha=NEG_SLOPE)
                        for h in range(HEADS):
                            # w = exp(.) straight into V's den column
                            nc.scalar.activation(
                                vv[:, :, 128 + h], az[:, h, 0:gl], ACTF.Exp)
                        nc.vector.tensor_tensor(
                            out=vv[:, :, 128:130], in0=vv[:, :, 128:130],
                            in1=msk_t[:, k0:k0 + gl].unsqueeze(2)
                                .broadcast_to([128, gl, 2]),
                            op=ALU.mult)
                        for h in range(HEADS):
                            nc.vector.tensor_tensor(
                                out=vv[:, :, h * HID:(h + 1) * HID],
                                in0=gv[:, :, h * HID:(h + 1) * HID],
                                in1=vv[:, :, 128 + h:129 + h]
                                    .broadcast_to([128, gl, HID]),
                                op=ALU.mult)
                        mm_rhs(psums, blk_of, k0, gl, vt, 130)

                for j in range(j0, j1):
                    psum = psums[j]
                    # +1e-30 keeps all-padding lanes finite (0 -> 1e30 ->
                    # 0*1e30=0)
                    dsafe = wp.tile([128, 2], F32, tag="dsafe")
                    nc.vector.tensor_scalar_add(dsafe[:], psum[:, 128:130],
                                                1e-30)
                    rden = wp.tile([128, 2], F32, tag="rden")
                    nc.vector.reciprocal(rden[:], dsafe[:])
                    h2pre = ep.tile([128, 128], BF, tag="h2pre")
                    for h in range(HEADS):
                        nc.vector.tensor_scalar(
                            out=h2pre[:, h * HID:(h + 1) * HID],
                            in0=psum[:, h * HID:(h + 1) * HID],
                            scalar1=rden[:, h:h + 1], scalar2=0.0,
                            op0=ALU.mult, op1=ALU.max)
                    tps = pst.tile([128, 128], BF, tag="tps")
                    nc.tensor.transpose(out=tps[:], in_=h2pre[:],
                                        identity=ident[:])
                    h2t = ep.tile([128, 128], BF, tag="h2t")
                    nc.scalar.copy(h2t[:], tps[:])
                    ps3 = psh.tile([128, 66], F32, tag="ps3", bufs=1)
                    nc.tensor.matmul(ps3[:], lhsT=h2t[:], rhs=w2_sb[:],
                                     start=True, stop=True)
                    h2row = hp.tile([128, 128], BF, tag="h2row")
                    nc.vector.tensor_tensor(out=h2row[:, 0:64],
                                            in0=ps3[:, 0:64],
                                            in1=b2_sb[:], op=ALU.add)
                    nc.scalar.copy(h2row[:, 64:65], ps3[:, 64:65])
                    nc.scalar.copy(adst2[:, j:j + 1], ps3[:, 65:66])
                    nc.sync.dma_start(out=h2_loc[j * 128:(j + 1) * 128, :],
                                      in_=h2row[:])

            def agg2_group(j0, j1):
                psums = {j: psa.tile([128, 65], F32, tag="ps",
                                     name=f"ps2_{j}")
                         for j in range(j0, j1)}
                for w in range(NW):
                    for (k0, gl, sub) in group_batches(j0, j1, w):
                        gt = gp.tile([128, gb * 128], BF, tag="g")
                        tab_ap = h2_full[wins[w][0]:wins[w][1], :]
                        nc.gpsimd.dma_gather(
                            gt[:, 0:gl * 128].rearrange(
                                "p (k c) -> p k c", c=128),
                            tab_ap,
                            idx_t[:, k0 * 8:(k0 + gl) * 8],
                            gl * 128, gl * 128, 128,
                            single_packet=False, queue_num=next_q(),
                        )
                        gv = gt[:, 0:gl * 128].rearrange(
                            "p (k c) -> p k c", c=128)
                        az = zp.tile([128, gb], F32, tag="az2")
                        vt = vp.tile([128, gb * 65], BF, tag="v2")
                        vv = vt[:, 0:gl * 65].rearrange(
                            "p (k c) -> p k c", c=65)
                        for (j, o0, ol) in sub:
                            nc.scalar.activation(
                                az[:, o0:o0 + ol], gv[:, o0:o0 + ol, 64],
                                ACTF.Prelu, bias=adst2[:, j:j + 1],
                                alpha=NEG_SLOPE)
                        nc.scalar.activation(vv[:, :, 64], az[:, 0:gl],
                                             ACTF.Exp)
                        nc.vector.tensor_tensor(
                            out=vv[:, :, 64:65], in0=vv[:, :, 64:65],
                            in1=msk_t[:, k0:k0 + gl].unsqueeze(2),
                            op=ALU.mult)
                        nc.vector.tensor_tensor(
                            out=vv[:, :, 0:64], in0=gv[:, :, 0:64],
                            in1=vv[:, :, 64:65].broadcast_to([128, gl, 64]),
                            op=ALU.mult)
                        mm_rhs(psums, blk_of, k0, gl, vt, 65)

                for j in range(j0, j1):
                    psum = psums[j]
                    dsafe = wp.tile([128, 1], F32, tag="dsafe2")
                    nc.vector.tensor_scalar_add(dsafe[:], psum[:, 64:65],
                                                1e-30)
                    rden = wp.tile([128, 1], F32, tag="rden2")
                    nc.vector.reciprocal(rden[:], dsafe[:])
                    ob = ep.tile([128, OUT], F32, tag="ob")
                    nc.scalar.activation(ob[:], psum[:, 0:64], ACTF.Sigmoid,
                                         scale=rden[:, 0:1])
                    nc.sync.dma_start(out=out_d[j * 128:(j + 1) * 128, :],
                                      in_=ob[:])

            for (j0, j1) in groups:
                agg1_group(j0, j1)
                if j1 in cuts:
                    ag_piece(h2_loc, h2_full, cuts.index(j1))
            for (j0, j1) in groups:
                agg2_group(j0, j1)

    nc.finalize()
    return nc


def kernel(x, edge_index, W1, att_src1, att_dst1, b1, W2, att_src2, att_dst2,
           b2):
    from concourse import bass_utils

    x = np.asarray(x, np.float32)
    W1 = np.asarray(W1, np.float32)
    W2 = np.asarray(W2, np.float32)
    att_src1 = np.asarray(att_src1, np.float32)
    att_dst1 = np.asarray(att_dst1, np.float32)
    att_src2 = np.asarray(att_src2, np.float32)
    att_dst2 = np.asarray(att_dst2, np.float32)
    b1 = np.asarray(b1, np.float32)
    b2 = np.asarray(b2, np.float32)
    n_nodes = x.shape[0]

    sch = _build_schedule(edge_index, n_nodes)
    vpad, pc = sch["vpad"], sch["pc"]

    W1r = W1.reshape(F_IN, HEADS, HID)
    w1_aug = np.zeros((F_IN, 130), np.float32)
    w1_aug[:, 0:HEADS * HID] = W1
    for h in range(HEADS):
        w1_aug[:, HEADS * HID + h] = W1r[:, h, :] @ att_dst1[h]
    w2_aug = np.zeros((HEADS * HID, 66), np.float32)
    w2_aug[:, 0:OUT] = W2
    w2_aug[:, OUT] = W2 @ att_src2[0]
    w2_aug[:, OUT + 1] = W2 @ att_dst2[0]
    b1_rep = np.broadcast_to(b1, (128, HEADS * HID)).copy()
    b2_rep = np.broadcast_to(b2, (128, OUT)).copy()
    asrc_rep = np.zeros((128, 128), np.float32)
    for h in range(HEADS):
        asrc_rep[:, h * HID:(h + 1) * HID] = att_src1[h]

    x_rho = np.zeros((vpad, F_IN), np.float32)
    x_rho[sch["row_of_node"]] = x

    key = (vpad, sch["nch"], tuple(sch["K"].tolist()),
           sch["Kw"], sch["cuts"])
    if key not in _cache:
        _cache[key] = _build_program(vpad, pc, sch["nb"], sch["wins"],
                                     sch["Kw"], sch["K"],
                                     sch["nch"], sch["cb"], sch["groups"],
                                     sch["cuts"])
    nc = _cache[key]

    in_maps = []
    for c in range(NCORES):
        in_maps.append({
            "xt": np.ascontiguousarray(
                x_rho[c * pc:(c + 1) * pc].T).astype(BF16),
            "idx": sch["idx_wrapped"][c],
            "msk": sch["mask_stream"][c],
            "w1aug": w1_aug.astype(BF16),
            "w2aug": w2_aug.astype(BF16),
            "b1rep": b1_rep,
            "b2rep": b2_rep,
            "asrcrep": asrc_rep.astype(BF16),
        })
    res = bass_utils.run_bass_kernel_spmd(nc, in_maps,
                                          core_ids=list(range(NCORES)),
                                          trace=TRACE)
    kernel.last_exec_ns = res.exec_time_ns
    kernel.last_mean_ns = res.mean_exec_time_ns
    kernel.last_res = res
    out_all = np.concatenate([res.results[c]["out"] for c in range(NCORES)], 0)
    return out_all[sch["row_of_node"][:n_nodes]]



# revision 16
# speedup vs baseline: 1.9082x; 1.9082x over previous
"""Two-layer GAT on 8 Trainium2 NeuronCores (Bass/Tile), bf16 gather path.

Host (numpy): append self-loops, degree-sort nodes (desc), pad node count to
VPAD (multiple of 8*128) and assign sorted nodes round-robin at 128-node
block granularity to the 8 cores (sorted-rank s -> block g=s//128,
lane=s%128 -> core c=g%8, local block j=g//8, table row = c*PC+j*128+lane).
Per block-rank j the chunk schedule is shared by all cores (SPMD: one
program, per-core tensor data).  Each dst node's edges occupy "slots"; a
chunk is slot k of all 128 lanes of a block.  Edge slots are split across
WINS_N overlapping 32768-row windows of the table (dma_gather indices are
int16; a left-to-right greedy balances each dst's edges over the windows
it is eligible for, minimizing the shared max-over-lanes chunk counts --
~1.25x padding vs 1.75x for a fixed half split).  Gather calls rotate
over the 4 SWDGE queues (4 concurrent Q7 descriptor generators, ~3x the
single-queue rate); per-call tiles are sized to the largest real call so
the gather pipeline can run 6 calls deep.

The gather table holds ONLY h (+bias) as 128 bf16 = 256B rows (the dma
gather minimum).  Per-edge alpha_src is recomputed on-chip from the
gathered h row via a DVE multiply + free-dim reduce against a replicated
a_src constant; alpha_dst stays a per-lane [128,1] activation bias
(lane-aligned slots).  Softmax denominators come from a tensor_reduce over
the per-block z tile instead of extra V columns, so the segment-sum matmul
V is a pure 128-col bf16 tile accumulated into PSUM with an identity lhsT.
x, W, the table, and V are all bf16; PSUM accumulation fp32.
"""

import numpy as np
import ml_dtypes

BF16 = ml_dtypes.bfloat16

NCORES = 8
F_IN = 128
HID = 64
HEADS = 2
OUT = 64
NEG_SLOPE = 0.2

GBATCH = 64  # max chunks per dma_gather
KMAX = 64    # z-tile slot capacity (assert K.max() <= KMAX)
PIECES = 2   # AllGather pieces (table row space is piece-major)
WINS_N = 3   # int16 gather windows over the table
GROUP = 1    # blocks whose window chunks share gather calls (= psum tiles
             # concurrently accumulating; bounded by PSUM banks)

TRACE = False
_cache = {}


def _build_schedule(edge_index, n_nodes):
    ei = np.asarray(edge_index).astype(np.int64)
    src = np.concatenate([ei[0], np.arange(n_nodes, dtype=np.int64)])
    dst = np.concatenate([ei[1], np.arange(n_nodes, dtype=np.int64)])
    deg = np.bincount(dst, minlength=n_nodes)

    stripe = NCORES * 128
    vpad = ((n_nodes + stripe - 1) // stripe) * stripe
    pc = vpad // NCORES
    nb = pc // 128
    assert vpad <= 3 * 32768

    # AllGather piece boundaries (in blocks); the gather-table row space is
    # piece-major (piece, core, block-within-piece, lane) so each piece's
    # collective output is one contiguous slice of h_full.
    cuts = sorted(set(min(nb, ((nb * (i + 1)) + PIECES - 1) // PIECES)
                      for i in range(PIECES)))
    starts = [0] + cuts[:-1]
    piece_of_j = np.zeros(nb, np.int64)
    tbase_of_j = np.zeros(nb, np.int64)
    cstr_of_j = np.zeros(nb, np.int64)
    joff_of_j = np.zeros(nb, np.int64)
    base = 0
    for p, (j0, j1) in enumerate(zip(starts, cuts)):
        bp = j1 - j0
        for j in range(j0, j1):
            piece_of_j[j] = p
            tbase_of_j[j] = base
            cstr_of_j[j] = bp * 128
            joff_of_j[j] = (j - j0) * 128
        base += NCORES * bp * 128
    assert base == vpad

    degp = np.zeros(vpad, np.int64)
    degp[:n_nodes] = deg
    order = np.argsort(-degp, kind="stable")
    rank = np.empty(vpad, np.int64)
    rank[order] = np.arange(vpad)

    s = np.arange(vpad)
    g = s // 128
    lane_r = s % 128
    c_r = g % NCORES
    j_r = g // NCORES
    local_of_rank = c_r * pc + j_r * 128 + lane_r
    trow_of_rank = (tbase_of_j[j_r] + c_r * cstr_of_j[j_r]
                    + joff_of_j[j_r] + lane_r)
    nrank = rank[:n_nodes]
    row_of_node = local_of_rank[nrank]       # for x placement / output
    trow_of_node = trow_of_rank[nrank]       # for gather indices
    c_of_node = c_r[nrank]
    j_of_node = j_r[nrank]
    lane_of_node = lane_r[nrank]

    e_srcrow = trow_of_node[src]
    e_c = c_of_node[dst]
    e_j = j_of_node[dst]
    e_lane = lane_of_node[dst]
    e_dkey = (e_c * nb + e_j) * 128 + e_lane  # dense (c, j, lane) id
    nkeys = NCORES * nb * 128

    # int16-addressable gather windows over the table rows; per-dst edges are
    # split across windows (balanced via the overlap regions) to minimize the
    # shared max-over-lanes chunk counts.  Left-to-right greedy: window w must
    # take every not-yet-assigned row below the next window's start, and takes
    # up to the block max of that forced count from its eligible range.
    W = 32768
    if vpad <= W:
        wins = [(0, vpad)]
    else:
        starts = [round((vpad - W) * i / (WINS_N - 1)) for i in range(WINS_N)]
        wins = [(st, st + W) for st in starts]
    NW = len(wins)

    def cnt(mask):
        return np.bincount(e_dkey[mask], minlength=nkeys)

    dcnt = cnt(np.ones(e_srcrow.shape[0], bool))
    jj = (np.arange(nkeys) // 128) % nb

    def blockmax(x):
        m = np.zeros(nb, np.int64)
        np.maximum.at(m, jj, x)
        return m

    assigned = np.zeros(nkeys, np.int64)
    Kw = []
    take_w = []
    for w in range(NW):
        nxt = wins[w + 1][0] if w + 1 < NW else vpad
        need = np.maximum(cnt(e_srcrow < nxt) - assigned, 0)
        K_w = blockmax(need)
        take = np.minimum(K_w[jj], cnt(e_srcrow < wins[w][1]) - assigned)
        take = np.maximum(take, need)
        assigned += take
        Kw.append(K_w)
        take_w.append(take)
    assert (assigned == dcnt).all()

    K = np.sum(Kw, axis=0)
    bump = K == 0
    Kw[0][bump] += 1
    K[bump] += 1

    # group-major chunk layout: per GROUP of blocks, per window, per block
    groups = [(j0, min(j0 + GROUP, nb)) for j0 in range(0, nb, GROUP)]
    cb = np.zeros((NW, nb), np.int64)
    cursor = 0
    for (j0, j1) in groups:
        for w in range(NW):
            for j in range(j0, j1):
                cb[w, j] = cursor
                cursor += int(Kw[w][j])
    nch = cursor
    assert nch == int(K.sum())

    # per-edge window: sort edges by (dst, src); window w takes the next
    # take_w[w] edges of each dst in src order
    ord_e = np.lexsort((e_srcrow, e_dkey))
    ds = e_dkey[ord_e]
    sr = e_srcrow[ord_e]
    first = np.r_[True, ds[1:] != ds[:-1]]
    grp_start = np.flatnonzero(first)
    grp_id = np.cumsum(first) - 1
    rank_in = np.arange(ds.shape[0]) - grp_start[grp_id]
    cumtake = np.cumsum(np.stack(take_w), axis=0)  # [NW, nkeys]
    ewin = (rank_in[:, None] >= cumtake[:, ds].T).sum(1)
    prev = np.where(ewin > 0,
                    cumtake[np.maximum(ewin - 1, 0), ds], 0)
    eslot = rank_in - prev
    j = (ds // 128) % nb
    ln = ds % 128
    c = ds // (nb * 128)
    pos = cb[ewin, j] + eslot
    assert (eslot >= 0).all()
    assert (eslot < np.stack([k[j] for k in Kw])[ewin,
                                                 np.arange(len(j))]).all()
    wlo = np.array([wins[w][0] for w in range(NW)])
    # every edge's src row must lie inside its window
    assert (sr >= wlo[ewin]).all()
    assert (sr < np.array([wins[w][1] for w in range(NW)])[ewin]).all()

    idx_val = sr - wlo[ewin]
    idx_stream = np.zeros((NCORES, 128, nch), np.int16)
    mask_stream = np.zeros((NCORES, 128, nch), BF16)
    idx_stream[c, ln, pos] = idx_val.astype(np.int16)
    mask_stream[c, ln, pos] = 1.0

    # wrapped int16 layout for dma_gather: chunk k -> columns 8k:8k+8 of
    # [128, 8*nch]; within a chunk the 128 lane-indices are wrapped as
    # flat[i] -> [i % 16, i // 16] and replicated over the 8 16-partition
    # groups.
    iw = idx_stream.transpose(0, 2, 1).reshape(NCORES, nch, 8, 16)
    iw = iw.transpose(0, 3, 1, 2).reshape(NCORES, 16, nch * 8)
    idx_wrapped = np.tile(iw, (1, 8, 1))

    return dict(vpad=vpad, pc=pc, nb=nb, wins=tuple(wins),
                Kw=tuple(tuple(k.tolist()) for k in Kw), K=K,
                nch=nch, cb=tuple(tuple(r.tolist()) for r in cb),
                groups=tuple(groups), row_of_node=row_of_node,
                cuts=tuple(cuts),
                idx_wrapped=np.ascontiguousarray(idx_wrapped),
                mask_stream=np.ascontiguousarray(mask_stream))


def _build_program(vpad, pc, nb, wins, Kw, K, nch, cb, groups, cuts):
    import concourse.bacc as bacc
    import concourse.bass as bass
    import concourse.mybir as mybir
    import concourse.tile as tile
    from concourse.masks import make_identity

    F32 = mybir.dt.float32
    BF = mybir.dt.bfloat16
    I16 = mybir.dt.int16
    ACTF = mybir.ActivationFunctionType
    ALU = mybir.AluOpType
    AXL = mybir.AxisListType

    NW = len(wins)
    firstq = {}
    lastq = {}
    for j in range(nb):
        ws = [w for w in range(NW) if Kw[w][j] > 0]
        firstq[j] = min(cb[w][j] for w in ws)
        lastq[j] = max(cb[w][j] + Kw[w][j] - 1 for w in ws)
    # largest single gather call (chunks): group batches are split at GBATCH
    gb = min(GBATCH, max(max(k) for k in Kw) * GROUP)

    nc = bacc.Bacc("TRN2", target_bir_lowering=False, debug=False,
                   num_devices=NCORES, num_swdge_queues=4)
    qn = [0]

    def next_q():
        q = qn[0]
        qn[0] = (q + 1) % 4
        return q

    xt_d = nc.dram_tensor("xt", [128, pc], BF, kind="ExternalInput")
    idx_d = nc.dram_tensor("idx", [128, nch * 8], I16, kind="ExternalInput")
    msk_d = nc.dram_tensor("msk", [128, nch], BF, kind="ExternalInput")
    w1_d = nc.dram_tensor("w1aug", [128, 130], BF, kind="ExternalInput")
    w2_d = nc.dram_tensor("w2aug", [128, 66], BF, kind="ExternalInput")
    b1_d = nc.dram_tensor("b1rep", [128, 128], F32, kind="ExternalInput")
    b2_d = nc.dram_tensor("b2rep", [128, 64], F32, kind="ExternalInput")
    as_d = nc.dram_tensor("asrcrep", [128, 128], BF, kind="ExternalInput")
    out_d = nc.dram_tensor("out", [pc, OUT], F32, kind="ExternalOutput")

    with tile.TileContext(nc) as tc:
        with (
            tc.tile_pool(name="const", bufs=1) as cp,
            tc.tile_pool(name="dram", bufs=1, space="DRAM") as dp,
            tc.tile_pool(name="hrow", bufs=3) as hp,
            tc.tile_pool(name="psh", bufs=2, space="PSUM") as psh,
            tc.tile_pool(name="g", bufs=6) as gp,
            tc.tile_pool(name="sc", bufs=2) as sp,
            tc.tile_pool(name="v", bufs=3) as vp,
            tc.tile_pool(name="z", bufs=3) as zp,
            tc.tile_pool(name="wz", bufs=4) as wp,
            tc.tile_pool(name="psa", bufs=4, space="PSUM") as psa,
            tc.tile_pool(name="pst", bufs=1, space="PSUM") as pst,
            tc.tile_pool(name="epi", bufs=3) as ep,
        ):
            ident = cp.tile([128, 128], BF)
            make_identity(nc, ident[:])
            w1_sb = cp.tile([128, 130], BF)
            w2_sb = cp.tile([128, 66], BF)
            b1_sb = cp.tile([128, 128], F32)
            b2_sb = cp.tile([128, 64], F32)
            as_sb = cp.tile([128, 128], BF)
            idx_t = cp.tile([128, nch * 8], I16)
            msk_t = cp.tile([128, nch], BF)
            xt_all = cp.tile([128, pc], BF)
            adst1 = cp.tile([128, 2 * nb], F32)
            adst2 = cp.tile([128, nb], F32)
            for t, d in ((w1_sb, w1_d), (w2_sb, w2_d), (b1_sb, b1_d),
                         (b2_sb, b2_d), (as_sb, as_d), (idx_t, idx_d),
                         (msk_t, msk_d), (xt_all, xt_d)):
                nc.sync.dma_start(out=t[:], in_=d[:])

            h_loc = dp.tile([pc, 128], BF)
            h2_loc = dp.tile([pc, 128], BF)
            h_full = dp.tile([vpad, 128], BF)
            h2_full = dp.tile([vpad, 128], BF)

            # piece p = blocks [starts[p], cuts[p]); its table rows are the
            # contiguous [tb, tb + 8*bp*128) slice (piece-major row space)
            starts = [0] + list(cuts[:-1])
            tbases = []
            tb = 0
            for j0, j1 in zip(starts, cuts):
                tbases.append(tb)
                tb += NCORES * (j1 - j0) * 128
            assert tb == vpad

            def ag_piece(loc, full, p):
                j0, j1 = starts[p], cuts[p]
                nc.gpsimd.collective_compute(
                    "AllGather", mybir.AluOpType.bypass,
                    replica_groups=[list(range(NCORES))],
                    ins=[loc[j0 * 128:j1 * 128, :]],
                    outs=[full[tbases[p]:tbases[p]
                               + NCORES * (j1 - j0) * 128, :]],
                )

            # ---- Phase 1: L1 h-phase (AllGather in overlapped pieces) ----
            for j in range(nb):
                ps = psh.tile([128, 130], F32, tag="psh")
                nc.tensor.matmul(ps[:], lhsT=xt_all[:, j * 128:(j + 1) * 128],
                                 rhs=w1_sb[:], start=True, stop=True)
                hrow = hp.tile([128, 128], BF, tag="hrow")
                nc.vector.tensor_tensor(out=hrow[:], in0=ps[:, 0:128],
                                        in1=b1_sb[:], op=ALU.add)
                nc.scalar.copy(adst1[:, j * 2:(j + 1) * 2], ps[:, 128:130])
                nc.sync.dma_start(out=h_loc[j * 128:(j + 1) * 128, :],
                                  in_=hrow[:])
                if j + 1 in cuts:
                    ag_piece(h_loc, h_full, cuts.index(j + 1))

            def group_batches(j0, j1, w):
                """Contiguous chunk batches (k0, gl, segs) of the group's
                window-w chunks; segs = [(j, call-offset, len), ...]."""
                segs = [(j, cb[w][j], Kw[w][j]) for j in range(j0, j1)
                        if Kw[w][j] > 0]
                if not segs:
                    return
                q0 = segs[0][1]
                qend = segs[-1][1] + segs[-1][2]
                b = q0
                while b < qend:
                    gl = min(gb, qend - b)
                    sub = []
                    for (j, s0, sl) in segs:
                        a = max(s0, b)
                        e = min(s0 + sl, b + gl)
                        if a < e:
                            sub.append((j, a - b, e - a))
                    yield b, gl, sub
                    b += gl

            def mm_rhs(psums, blk_of, k0, gl, vt, vcols):
                for k in range(gl):
                    q = k0 + k
                    j = blk_of[q]
                    nc.tensor.matmul(
                        psums[j][:], lhsT=ident[:],
                        rhs=vt[:, k * vcols:(k + 1) * vcols],
                        start=(q == firstq[j]), stop=(q == lastq[j]))

            blk_of = np.zeros(nch, np.int64)
            for j in range(nb):
                for w in range(NW):
                    blk_of[cb[w][j]:cb[w][j] + Kw[w][j]] = j

            def agg1_group(j0, j1):
                psums = {j: psa.tile([128, 130], F32, tag="ps",
                                     name=f"ps1_{j}")
                         for j in range(j0, j1)}
                for w in range(NW):
                    for (k0, gl, sub) in group_batches(j0, j1, w):
                        gt = gp.tile([128, gb * 128], BF, tag="g")
                        tab_ap = h_full[wins[w][0]:wins[w][1], :]
                        nc.gpsimd.dma_gather(
                            gt[:, 0:gl * 128].rearrange(
                                "p (k c) -> p k c", c=128),
                            tab_ap,
                            idx_t[:, k0 * 8:(k0 + gl) * 8],
                            gl * 128, gl * 128, 128,
                            single_packet=False, queue_num=next_q(),
                        )
                        gv = gt[:, 0:gl * 128].rearrange(
                            "p (k c) -> p k c", c=128)
                        az = zp.tile([128, 2, gb], F32, tag="az")
                        vt = vp.tile([128, gb * 130], BF, tag="v")
                        vv = vt[:, 0:gl * 130].rearrange(
                            "p (k c) -> p k c", c=130)
                        # alpha_src per edge: dot(h_row, a_src) per head,
                        # both heads in one mult + one reduce
                        scr = sp.tile([128, gb * 128], BF, tag="s")
                        sv = scr[:, 0:gl * 128].rearrange(
                            "p (k h c) -> p k h c", h=2, c=HID)
                        nc.vector.tensor_tensor(
                            out=sv,
                            in0=gv.rearrange("p k (h c) -> p k h c", c=HID),
                            in1=as_sb[:].rearrange("p (h c) -> p h c", c=HID)
                                .unsqueeze(1).broadcast_to([128, gl, 2, HID]),
                            op=ALU.mult)
                        azr = az[:, :, :].rearrange("p h k -> p k h")
                        nc.vector.tensor_reduce(
                            out=azr[:, 0:gl, :], in_=sv,
                            axis=AXL.X, op=ALU.add)
                        for (j, o0, ol) in sub:
                            for h in range(HEADS):
                                nc.scalar.activation(
                                    az[:, h, o0:o0 + ol], az[:, h, o0:o0 + ol],
                                    ACTF.Prelu,
                                    bias=adst1[:, 2 * j + h:2 * j + h + 1],
                                    alpha=NEG_SLOPE)
                        for h in range(HEADS):
                            # w = exp(.) straight into V's den column
                            nc.scalar.activation(
                                vv[:, :, 128 + h], az[:, h, 0:gl], ACTF.Exp)
                        nc.vector.tensor_tensor(
                            out=vv[:, :, 128:130], in0=vv[:, :, 128:130],
                            in1=msk_t[:, k0:k0 + gl].unsqueeze(2)
                                .broadcast_to([128, gl, 2]),
                            op=ALU.mult)
                        for h in range(HEADS):
                            nc.vector.tensor_tensor(
                                out=vv[:, :, h * HID:(h + 1) * HID],
                                in0=gv[:, :, h * HID:(h + 1) * HID],
                                in1=vv[:, :, 128 + h:129 + h]
                                    .broadcast_to([128, gl, HID]),
                                op=ALU.mult)
                        mm_rhs(psums, blk_of, k0, gl, vt, 130)

                for j in range(j0, j1):
                    psum = psums[j]
                    # +1e-30 keeps all-padding lanes finite (0 -> 1e30 ->
                    # 0*1e30=0)
                    dsafe = wp.tile([128, 2], F32, tag="dsafe")
                    nc.vector.tensor_scalar_add(dsafe[:], psum[:, 128:130],
                                                1e-30)
                    rden = wp.tile([128, 2], F32, tag="rden")
                    nc.vector.reciprocal(rden[:], dsafe[:])
                    h2pre = ep.tile([128, 128], BF, tag="h2pre")
                    for h in range(HEADS):
                        nc.vector.tensor_scalar(
                            out=h2pre[:, h * HID:(h + 1) * HID],
                            in0=psum[:, h * HID:(h + 1) * HID],
                            scalar1=rden[:, h:h + 1], scalar2=0.0,
                            op0=ALU.mult, op1=ALU.max)
                    tps = pst.tile([128, 128], BF, tag="tps")
                    nc.tensor.transpose(out=tps[:], in_=h2pre[:],
                                        identity=ident[:])
                    h2t = ep.tile([128, 128], BF, tag="h2t")
                    nc.scalar.copy(h2t[:], tps[:])
                    ps3 = psh.tile([128, 66], F32, tag="ps3", bufs=1)
                    nc.tensor.matmul(ps3[:], lhsT=h2t[:], rhs=w2_sb[:],
                                     start=True, stop=True)
                    h2row = hp.tile([128, 128], BF, tag="h2row")
                    nc.vector.tensor_tensor(out=h2row[:, 0:64],
                                            in0=ps3[:, 0:64],
                                            in1=b2_sb[:], op=ALU.add)
                    nc.scalar.copy(h2row[:, 64:65], ps3[:, 64:65])
                    nc.scalar.copy(adst2[:, j:j + 1], ps3[:, 65:66])
                    nc.sync.dma_start(out=h2_loc[j * 128:(j + 1) * 128, :],
                                      in_=h2row[:])

            def agg2_group(j0, j1):
                psums = {j: psa.tile([128, 65], F32, tag="ps",
                                     name=f"ps2_{j}")
                         for j in range(j0, j1)}
                for w in range(NW):
                    for (k0, gl, sub) in group_batches(j0, j1, w):
                        gt = gp.tile([128, gb * 128], BF, tag="g")
                        tab_ap = h2_full[wins[w][0]:wins[w][1], :]
                        nc.gpsimd.dma_gather(
                            gt[:, 0:gl * 128].rearrange(
                                "p (k c) -> p k c", c=128),
                            tab_ap,
                            idx_t[:, k0 * 8:(k0 + gl) * 8],
                            gl * 128, gl * 128, 128,
                            single_packet=False, queue_num=next_q(),
                        )
                        gv = gt[:, 0:gl * 128].rearrange(
                            "p (k c) -> p k c", c=128)
                        az = zp.tile([128, gb], F32, tag="az2")
                        vt = vp.tile([128, gb * 65], BF, tag="v2")
                        vv = vt[:, 0:gl * 65].rearrange(
                            "p (k c) -> p k c", c=65)
                        for (j, o0, ol) in sub:
                            nc.scalar.activation(
                                az[:, o0:o0 + ol], gv[:, o0:o0 + ol, 64],
                                ACTF.Prelu, bias=adst2[:, j:j + 1],
                                alpha=NEG_SLOPE)
                        nc.scalar.activation(vv[:, :, 64], az[:, 0:gl],
                                             ACTF.Exp)
                        nc.vector.tensor_tensor(
                            out=vv[:, :, 64:65], in0=vv[:, :, 64:65],
                            in1=msk_t[:, k0:k0 + gl].unsqueeze(2),
                            op=ALU.mult)
                        nc.vector.tensor_tensor(
                            out=vv[:, :, 0:64], in0=gv[:, :, 0:64],
                            in1=vv[:, :, 64:65].broadcast_to([128, gl, 64]),
                            op=ALU.mult)
                        mm_rhs(psums, blk_of, k0, gl, vt, 65)

                for j in range(j0, j1):
                    psum = psums[j]
                    dsafe = wp.tile([128, 1], F32, tag="dsafe2")
                    nc.vector.tensor_scalar_add(dsafe[:], psum[:, 64:65],
                                                1e-30)
                    rden = wp.tile([128, 1], F32, tag="rden2")
                    nc.vector.reciprocal(rden[:], dsafe[:])
                    ob = ep.tile([128, OUT], F32, tag="ob")
                    nc.scalar.activation(ob[:], psum[:, 0:64], ACTF.Sigmoid,
                                         scale=rden[:, 0:1])
                    nc.sync.dma_start(out=out_d[j * 128:(j + 1) * 128, :],
                                      in_=ob[:])

            for (j0, j1) in groups:
                agg1_group(j0, j1)
                if j1 in cuts:
                    ag_piece(h2_loc, h2_full, cuts.index(j1))
            for (j0, j1) in groups:
                agg2_group(j0, j1)

    nc.finalize()
    return nc


def kernel(x, edge_index, W1, att_src1, att_dst1, b1, W2, att_src2, att_dst2,
           b2):
    from concourse import bass_utils

    x = np.asarray(x, np.float32)
    W1 = np.asarray(W1, np.float32)
    W2 = np.asarray(W2, np.float32)
    att_src1 = np.asarray(att_src1, np.float32)
    att_dst1 = np.asarray(att_dst1, np.float32)
    att_src2 = np.asarray(att_src2, np.float32)
    att_dst2 = np.asarray(att_dst2, np.float32)
    b1 = np.asarray(b1, np.float32)
    b2 = np.asarray(b2, np.float32)
    n_nodes = x.shape[0]

    sch = _build_schedule(edge_index, n_nodes)
    vpad, pc = sch["vpad"], sch["pc"]

    W1r = W1.reshape(F_IN, HEADS, HID)
    w1_aug = np.zeros((F_IN, 130), np.float32)
    w1_aug[:, 0:HEADS * HID] = W1
    for h in range(HEADS):
        w1_aug[:, HEADS * HID + h] = W1r[:, h, :] @ att_dst1[h]
    w2_aug = np.zeros((HEADS * HID, 66), np.float32)
    w2_aug[:, 0:OUT] = W2
    w2_aug[:, OUT] = W2 @ att_src2[0]
    w2_aug[:, OUT + 1] = W2 @ att_dst2[0]
    b1_rep = np.broadcast_to(b1, (128, HEADS * HID)).copy()
    b2_rep = np.broadcast_to(b2, (128, OUT)).copy()
    asrc_rep = np.zeros((128, 128), np.float32)
    for h in range(HEADS):
        asrc_rep[:, h * HID:(h + 1) * HID] = att_src1[h]

    x_rho = np.zeros((vpad, F_IN), np.float32)
    x_rho[sch["row_of_node"]] = x

    key = (vpad, sch["nch"], tuple(sch["K"].tolist()),
           sch["Kw"], sch["cuts"])
    if key not in _cache:
        _cache[key] = _build_program(vpad, pc, sch["nb"], sch["wins"],
                                     sch["Kw"], sch["K"],
                                     sch["nch"], sch["cb"], sch["groups"],
                                     sch["cuts"])
    nc = _cache[key]

    in_maps = []
    for c in range(NCORES):
        in_maps.append({
            "xt": np.ascontiguousarray(
                x_rho[c * pc:(c + 1) * pc].T).astype(BF16),
            "idx": sch["idx_wrapped"][c],
            "msk": sch["mask_stream"][c],
            "w1aug": w1_aug.astype(BF16),
            "w2aug": w2_aug.astype(BF16),
            "b1rep": b1_rep,
            "b2rep": b2_rep,
            "asrcrep": asrc_rep.astype(BF16),
        })
    res = bass_utils.run_bass_kernel_spmd(nc, in_maps,
                                          core_ids=list(range(NCORES)),
                                          trace=TRACE)
    kernel.last_exec_ns = res.exec_time_ns
    kernel.last_mean_ns = res.mean_exec_time_ns
    kernel.last_res = res
    out_all = np.concatenate([res.results[c]["out"] for c in range(NCORES)], 0)
    return out_all[sch["row_of_node"][:n_nodes]]



# revision 18
# speedup vs baseline: 2.1232x; 1.1127x over previous
"""Two-layer GAT on 8 TRN2 cores: single-window signed-idx gather V2."""
import os
import numpy as np
import ml_dtypes

BF16 = ml_dtypes.bfloat16

NCORES = 8
F_IN = 128
HID = 64
HEADS = 2
OUT = 64
NEG_SLOPE = 0.2

GB = 8
NPIECES = 4

TRACE = False
_cache = {}


def _build_schedule(edge_index, n_nodes):
    ei = np.asarray(edge_index).astype(np.int64)
    src, dst = ei[0], ei[1]
    deg = np.bincount(dst, minlength=n_nodes)

    stripe = NCORES * 128
    vpad = ((n_nodes + stripe - 1) // stripe) * stripe
    pc = vpad // NCORES
    nb = pc // 128
    bias = max(0, vpad - 32768)
    assert vpad - bias <= 32768 and bias <= 32768

    degp = np.zeros(vpad, np.int64)
    degp[:n_nodes] = deg
    order = np.argsort(-degp, kind="stable")
    rank = np.empty(vpad, np.int64)
    rank[order] = np.arange(vpad)

    s = np.arange(vpad)
    g = s // 128
    lane_r = s % 128
    c_r = g % NCORES
    j_r = g // NCORES

    cuts = sorted(set(min(nb, ((nb * (p + 1)) + NPIECES - 1) // NPIECES)
                      for p in range(NPIECES)))
    starts = [0] + cuts[:-1]
    tb = 0
    tbase_of_j = np.zeros(nb, np.int64)
    cstr_of_j = np.zeros(nb, np.int64)
    joff_of_j = np.zeros(nb, np.int64)
    tbases = []
    for j0, j1 in zip(starts, cuts):
        bp = j1 - j0
        tbases.append(tb)
        for j in range(j0, j1):
            tbase_of_j[j] = tb
            cstr_of_j[j] = bp * 128
            joff_of_j[j] = (j - j0) * 128
        tb += NCORES * bp * 128
    assert tb == vpad

    trow_of_rank = (tbase_of_j[j_r] + c_r * cstr_of_j[j_r]
                    + joff_of_j[j_r] + lane_r)
    local_of_rank = c_r * pc + j_r * 128 + lane_r
    nrank = rank[:n_nodes]
    row_of_node = local_of_rank[nrank]
    trow_of_node = trow_of_rank[nrank]

    e_srcrow = trow_of_node[src]
    e_c = c_r[nrank][dst]
    e_j = j_r[nrank][dst]
    e_lane = lane_r[nrank][dst]
    e_dkey = (e_c * nb + e_j) * 128 + e_lane
    nkeys = NCORES * nb * 128

    cnt = np.bincount(e_dkey, minlength=nkeys).reshape(NCORES, nb, 128)
    K = cnt.max(axis=(0, 2))
    cb = np.zeros(nb + 1, np.int64)
    cb[1:] = np.cumsum(K)
    nch = int(cb[-1])

    ord_e = np.lexsort((e_srcrow, e_dkey))
    ds = e_dkey[ord_e]
    first = np.r_[True, ds[1:] != ds[:-1]]
    grp_start = np.flatnonzero(first)
    grp_id = np.cumsum(first) - 1
    rank_in = np.arange(ds.shape[0]) - grp_start[grp_id]
    j_e = (ds // 128) % nb
    ln = ds % 128
    c = ds // (nb * 128)
    pos = cb[j_e] + rank_in
    assert (rank_in < K[j_e]).all()

    idx_stream = np.zeros((NCORES, 128, nch), np.int16)
    mask_stream = np.zeros((NCORES, 128, nch), BF16)
    idx_stream[c, ln, pos] = (e_srcrow[ord_e] - bias).astype(np.int16)
    mask_stream[c, ln, pos] = 1.0

    batches = []
    for j in range(nb):
        b = int(cb[j])
        while b < cb[j + 1]:
            gl = int(min(GB, cb[j + 1] - b))
            batches.append((b, gl))
            b += gl
    # ucode trims trailing idx<0 per call: force lane 127's entry at each
    # call's last chunk non-negative via an in-lane slot swap.  Donor slots
    # must exclude every call's qlast position, else a later swap steals a
    # previously fixed entry (edges are src-sorted, so negatives cluster at
    # early slots and multiple batches would reuse the same donor).
    qlast_set = set(b0 + gl - 1 for (b0, gl) in batches)
    for (b0, gl) in batches:
        qlast = b0 + gl - 1
        j = int(np.searchsorted(cb, qlast, side="right") - 1)
        for cc in range(NCORES):
            if idx_stream[cc, 127, qlast] < 0:
                sl = np.array([q for q in range(int(cb[j]), int(cb[j + 1]))
                               if q not in qlast_set])
                cand = sl[idx_stream[cc, 127, sl] >= 0]
                assert cand.size > 0, "no non-negative donor for lane 127"
                s2 = int(cand[-1])
                for arr in (idx_stream, mask_stream):
                    t = arr[cc, 127, qlast].copy()
                    arr[cc, 127, qlast] = arr[cc, 127, s2]
                    arr[cc, 127, s2] = t
    for (b0, gl) in batches:
        assert (idx_stream[:, 127, b0 + gl - 1] >= 0).all()

    iw = idx_stream.transpose(0, 2, 1).reshape(NCORES, nch, 8, 16)
    iw = iw.transpose(0, 3, 1, 2).reshape(NCORES, 16, nch * 8)
    idx_wrapped = np.tile(iw, (1, 8, 1))

    return dict(vpad=vpad, pc=pc, nb=nb, K=tuple(K.tolist()), nch=nch,
                bias=bias, cuts=tuple(cuts), tbases=tuple(tbases),
                batches=tuple(batches), row_of_node=row_of_node,
                idx_wrapped=np.ascontiguousarray(idx_wrapped),
                mask_stream=np.ascontiguousarray(mask_stream))


def _build_program(vpad, pc, nb, K, nch, bias, cuts, tbases, batches):
    import concourse.bacc as bacc
    import concourse.mybir as mybir
    import concourse.tile as tile
    from concourse.masks import make_identity

    F32 = mybir.dt.float32
    BF = mybir.dt.bfloat16
    I16 = mybir.dt.int16
    ACTF = mybir.ActivationFunctionType
    PRELU = (ACTF.Relu if os.environ.get('KSIMRELU') == '1' else ACTF.Prelu)
    ALU = mybir.AluOpType
    AXL = mybir.AxisListType

    starts = [0] + list(cuts[:-1])
    cb = np.zeros(nb + 1, np.int64)
    cb[1:] = np.cumsum(np.array(K))
    blk_of = np.zeros(max(nch, 1), np.int64)
    for j in range(nb):
        blk_of[cb[j]:cb[j + 1]] = j
    firstq = {j: int(cb[j]) for j in range(nb) if K[j] > 0}
    lastq = {j: int(cb[j + 1] - 1) for j in range(nb) if K[j] > 0}

    nc = bacc.Bacc("TRN2", target_bir_lowering=False, debug=False,
                   num_devices=NCORES, num_swdge_queues=4)
    qn = [0]

    def next_q():
        q = qn[0]
        qn[0] = (q + 1) % 4
        return q

    xt_d = nc.dram_tensor("xt", [128, pc], BF, kind="ExternalInput")
    idx_d = nc.dram_tensor("idx", [128, nch * 8], I16, kind="ExternalInput")
    msk_d = nc.dram_tensor("msk", [128, nch], BF, kind="ExternalInput")
    w1_d = nc.dram_tensor("w1aug", [128, 132], BF, kind="ExternalInput")
    w2_d = nc.dram_tensor("w2aug", [128, 66], BF, kind="ExternalInput")
    b1_d = nc.dram_tensor("b1rep", [128, 128], F32, kind="ExternalInput")
    b2_d = nc.dram_tensor("b2rep", [128, 64], F32, kind="ExternalInput")
    as_d = nc.dram_tensor("asrcrep", [128, 128], BF, kind="ExternalInput")
    out_d = nc.dram_tensor("out", [pc, OUT], F32, kind="ExternalOutput")

    with tile.TileContext(nc) as tc:
        with (
            tc.tile_pool(name="const", bufs=1) as cp,
            tc.tile_pool(name="dram", bufs=1, space="DRAM") as dp,
            tc.tile_pool(name="hrow", bufs=3) as hp,
            tc.tile_pool(name="psh", bufs=2, space="PSUM") as psh,
            tc.tile_pool(name="g", bufs=8) as gp,
            tc.tile_pool(name="sc", bufs=2) as sp,
            tc.tile_pool(name="v", bufs=3) as vp,
            tc.tile_pool(name="z", bufs=3) as zp,
            tc.tile_pool(name="wz", bufs=4) as wp,
            tc.tile_pool(name="psa", bufs=4, space="PSUM") as psa,
            tc.tile_pool(name="pst", bufs=1, space="PSUM") as pst,
            tc.tile_pool(name="epi", bufs=3) as ep,
        ):
            ident = cp.tile([128, 128], BF)
            make_identity(nc, ident[:])
            w1_sb = cp.tile([128, 132], BF)
            w2_sb = cp.tile([128, 66], BF)
            b1_sb = cp.tile([128, 128], F32)
            b2_sb = cp.tile([128, 64], F32)
            as_sb = cp.tile([128, 128], BF)
            idx_t = cp.tile([128, nch * 8], I16)
            msk_t = cp.tile([128, nch], BF)
            xt_all = cp.tile([128, pc], BF)
            adst1 = cp.tile([128, 2 * nb], F32)
            adst2 = cp.tile([128, nb], F32)
            vself = cp.tile([128, nb * 130], BF)
            v2self = cp.tile([128, nb * 65], BF)
            fence1 = cp.tile([128, 1], BF)
            fence2 = cp.tile([128, 1], BF)
            for t, d in ((w1_sb, w1_d), (w2_sb, w2_d), (b1_sb, b1_d),
                         (b2_sb, b2_d), (as_sb, as_d), (idx_t, idx_d),
                         (msk_t, msk_d), (xt_all, xt_d)):
                nc.sync.dma_start(out=t[:], in_=d[:])

            h_loc = dp.tile([pc, 128], BF)
            h2_loc = dp.tile([pc, 128], BF)
            h_full = dp.tile([vpad, 128], BF)
            h2_full = dp.tile([vpad, 128], BF)

            def ag_piece(loc, full, p):
                j0, j1 = starts[p], cuts[p]
                nc.gpsimd.collective_compute(
                    "AllGather", mybir.AluOpType.bypass,
                    replica_groups=[list(range(NCORES))],
                    ins=[loc[j0 * 128:j1 * 128, :]],
                    outs=[full[tbases[p]:tbases[p]
                               + NCORES * (j1 - j0) * 128, :]],
                )

            # ---- L1 h-phase ----
            for j in range(nb):
                ps = psh.tile([128, 132], F32, tag="psh")
                nc.tensor.matmul(ps[:], lhsT=xt_all[:, j * 128:(j + 1) * 128],
                                 rhs=w1_sb[:], start=True, stop=True)
                hrow = hp.tile([128, 128], BF, tag="hrow")
                nc.vector.tensor_tensor(out=hrow[:], in0=ps[:, 0:128],
                                        in1=b1_sb[:], op=ALU.add)
                nc.scalar.copy(adst1[:, j * 2:(j + 1) * 2], ps[:, 128:130])
                wsf = wp.tile([128, 2], F32, tag="wsf")
                nc.vector.tensor_tensor(out=wsf[:], in0=ps[:, 130:132],
                                        in1=adst1[:, j * 2:(j + 1) * 2],
                                        op=ALU.add)
                nc.scalar.activation(wsf[:], wsf[:], PRELU, alpha=NEG_SLOPE)
                wsb = wp.tile([128, 2], BF, tag="wsb")
                nc.scalar.activation(wsb[:], wsf[:], ACTF.Exp)
                vs = vself[:, j * 130:(j + 1) * 130]
                nc.vector.tensor_tensor(
                    out=vs[:, 0:128].rearrange("p (h c) -> p h c", c=HID),
                    in0=hrow[:].rearrange("p (h c) -> p h c", c=HID),
                    in1=wsb[:].unsqueeze(2).broadcast_to([128, 2, HID]),
                    op=ALU.mult)
                nc.scalar.copy(vs[:, 128:130], wsb[:])
                nc.sync.dma_start(out=h_loc[j * 128:(j + 1) * 128, :],
                                  in_=hrow[:])
                if j + 1 in cuts:
                    ag_piece(h_loc, h_full, cuts.index(j + 1))

            # pieces fully below `bias` aren't covered by the gather base
            # AP; route idx through a bypass reading the piece-0 region so
            # the tile framework orders gathers after those AllGathers
            uncovered = [p for p, (j0, j1) in enumerate(zip(starts, cuts))
                         if tbases[p] + NCORES * (j1 - j0) * 128 <= bias]

            def make_fence(full, fence):
                nc.sync.dma_start(out=fence[:], in_=full[0:128, 0:1])
                for _ in range(8):
                    t = gp.tile([128, GB * 128], BF, tag="g")
                    nc.vector.tensor_scalar(
                        out=t[:, 0:1], in0=fence[:], scalar1=0.0,
                        scalar2=0.0, op0=ALU.mult, op1=ALU.add)

            if uncovered:
                assert uncovered == [0], uncovered
                make_fence(h_full, fence1)

            pend_ag = []

            def seg_list(b0, gl):
                segs = []
                for q in range(b0, b0 + gl):
                    j = int(blk_of[q])
                    if segs and segs[-1][0] == j:
                        segs[-1][2] += 1
                    else:
                        segs.append([j, q - b0, 1])
                return segs

            def epilogue1(j, psum):
                rden = wp.tile([128, 2], F32, tag="rden")
                nc.vector.reciprocal(rden[:], psum[:, 128:130])
                h2pre = ep.tile([128, 128], BF, tag="h2pre")
                for h in range(HEADS):
                    nc.vector.tensor_scalar(
                        out=h2pre[:, h * HID:(h + 1) * HID],
                        in0=psum[:, h * HID:(h + 1) * HID],
                        scalar1=rden[:, h:h + 1], scalar2=0.0,
                        op0=ALU.mult, op1=ALU.max)
                tps = pst.tile([128, 128], BF, tag="tps")
                nc.tensor.transpose(out=tps[:], in_=h2pre[:],
                                    identity=ident[:])
                h2t = ep.tile([128, 128], BF, tag="h2t")
                nc.scalar.copy(h2t[:], tps[:])
                ps3f = psh.tile([128, 132], F32, tag="psh")
                nc.tensor.matmul(ps3f[:, 0:66], lhsT=h2t[:], rhs=w2_sb[:],
                                 start=True, stop=True)
                h2row = hp.tile([128, 128], BF, tag="h2row")
                nc.vector.tensor_tensor(out=h2row[:, 0:64],
                                        in0=ps3f[:, 0:64],
                                        in1=b2_sb[:], op=ALU.add)
                nc.scalar.copy(h2row[:, 64:65], ps3f[:, 64:65])
                nc.scalar.copy(adst2[:, j:j + 1], ps3f[:, 65:66])
                ws2f = wp.tile([128, 1], F32, tag="ws2f")
                nc.vector.tensor_tensor(out=ws2f[:], in0=ps3f[:, 64:65],
                                        in1=adst2[:, j:j + 1], op=ALU.add)
                nc.scalar.activation(ws2f[:], ws2f[:], PRELU,
                                     alpha=NEG_SLOPE)
                ws2b = wp.tile([128, 1], BF, tag="ws2b")
                nc.scalar.activation(ws2b[:], ws2f[:], ACTF.Exp)
                v2 = v2self[:, j * 65:(j + 1) * 65]
                nc.vector.tensor_tensor(
                    out=v2[:, 0:64], in0=h2row[:, 0:64],
                    in1=ws2b[:].broadcast_to([128, 64]), op=ALU.mult)
                nc.scalar.copy(v2[:, 64:65], ws2b[:])
                nc.vector.memset(h2row[:, 65:128], 0.0)
                nc.sync.dma_start(out=h2_loc[j * 128:(j + 1) * 128, :],
                                  in_=h2row[:])
                if j + 1 in cuts:
                    p = cuts.index(j + 1)
                    pend_ag.append(lambda p=p: ag_piece(h2_loc, h2_full, p))

            # ---- L1 aggregation stream ----
            psums = {}
            for (b0, gl) in batches:
                gt = gp.tile([128, GB * 128], BF, tag="g")
                nc.gpsimd.dma_gather(
                    gt[:, 0:gl * 128].rearrange("p (k c) -> p k c", c=128),
                    h_full[bias:vpad, :],
                    idx_t[:, b0 * 8:(b0 + gl) * 8],
                    gl * 128, gl * 128, 128,
                    single_packet=False, queue_num=next_q(),
                )
                while pend_ag:
                    pend_ag.pop(0)()
                gv = gt[:, 0:gl * 128].rearrange("p (k c) -> p k c", c=128)
                az = zp.tile([128, 2, GB], F32, tag="az")
                vt = vp.tile([128, GB * 130], BF, tag="v")
                vv = vt[:, 0:gl * 130].rearrange("p (k c) -> p k c", c=130)
                scr = sp.tile([128, GB * 128], BF, tag="s")
                sv = scr[:, 0:gl * 128].rearrange(
                    "p (k h c) -> p k h c", h=2, c=HID)
                nc.vector.tensor_tensor(
                    out=sv,
                    in0=gv.rearrange("p k (h c) -> p k h c", c=HID),
                    in1=as_sb[:].rearrange("p (h c) -> p h c", c=HID)
                        .unsqueeze(1).broadcast_to([128, gl, 2, HID]),
                    op=ALU.mult)
                azr = az[:, :, :].rearrange("p h k -> p k h")
                nc.vector.tensor_reduce(
                    out=azr[:, 0:gl, :], in_=sv, axis=AXL.X, op=ALU.add)
                for (j, o0, ol) in seg_list(b0, gl):
                    for h in range(HEADS):
                        nc.scalar.activation(
                            az[:, h, o0:o0 + ol], az[:, h, o0:o0 + ol],
                            PRELU,
                            bias=adst1[:, 2 * j + h:2 * j + h + 1],
                            alpha=NEG_SLOPE)
                for h in range(HEADS):
                    nc.scalar.activation(
                        vv[:, :, 128 + h], az[:, h, 0:gl], ACTF.Exp)
                nc.vector.tensor_tensor(
                    out=vv[:, :, 128:130], in0=vv[:, :, 128:130],
                    in1=msk_t[:, b0:b0 + gl].unsqueeze(2)
                        .broadcast_to([128, gl, 2]),
                    op=ALU.mult)
                for h in range(HEADS):
                    nc.vector.tensor_tensor(
                        out=vv[:, :, h * HID:(h + 1) * HID],
                        in0=gv[:, :, h * HID:(h + 1) * HID],
                        in1=vv[:, :, 128 + h:129 + h]
                            .broadcast_to([128, gl, HID]),
                        op=ALU.mult)
                for k in range(gl):
                    q = b0 + k
                    j = int(blk_of[q])
                    if q == firstq[j]:
                        psums[j] = psa.tile([128, 130], F32, tag="ps",
                                            name=f"ps1_{j}")
                        nc.tensor.matmul(psums[j][:], lhsT=ident[:],
                                         rhs=vself[:, j * 130:(j + 1) * 130],
                                         start=True, stop=False)
                    nc.tensor.matmul(
                        psums[j][:], lhsT=ident[:],
                        rhs=vt[:, k * 130:(k + 1) * 130],
                        start=False, stop=(q == lastq[j]))
                    if q == lastq[j]:
                        epilogue1(j, psums.pop(j))

            for j in range(nb):
                if K[j] == 0:
                    psum = psa.tile([128, 130], F32, tag="ps",
                                    name=f"ps1_{j}")
                    nc.tensor.matmul(psum[:], lhsT=ident[:],
                                     rhs=vself[:, j * 130:(j + 1) * 130],
                                     start=True, stop=True)
                    epilogue1(j, psum)
            while pend_ag:
                pend_ag.pop(0)()

            if uncovered:
                make_fence(h2_full, fence2)

            # ---- L2 aggregation stream ----
            def epilogue2(j, psum):
                rden = wp.tile([128, 1], F32, tag="rden2")
                nc.vector.reciprocal(rden[:], psum[:, 64:65])
                ob = ep.tile([128, OUT], F32, tag="ob")
                nc.scalar.activation(ob[:], psum[:, 0:64], ACTF.Sigmoid,
                                     scale=rden[:, 0:1])
                nc.sync.dma_start(out=out_d[j * 128:(j + 1) * 128, :],
                                  in_=ob[:])

            psums2 = {}
            for (b0, gl) in batches:
                gt = gp.tile([128, GB * 128], BF, tag="g")
                nc.gpsimd.dma_gather(
                    gt[:, 0:gl * 128].rearrange("p (k c) -> p k c", c=128),
                    h2_full[bias:vpad, :],
                    idx_t[:, b0 * 8:(b0 + gl) * 8],
                    gl * 128, gl * 128, 128,
                    single_packet=False, queue_num=next_q(),
                )
                gv = gt[:, 0:gl * 128].rearrange("p (k c) -> p k c", c=128)
                az = zp.tile([128, GB], F32, tag="az2")
                vt = vp.tile([128, GB * 65], BF, tag="v2")
                vv = vt[:, 0:gl * 65].rearrange("p (k c) -> p k c", c=65)
                for (j, o0, ol) in seg_list(b0, gl):
                    nc.scalar.activation(
                        az[:, o0:o0 + ol], gv[:, o0:o0 + ol, 64],
                        PRELU, bias=adst2[:, j:j + 1],
                        alpha=NEG_SLOPE)
                nc.scalar.activation(vv[:, :, 64], az[:, 0:gl], ACTF.Exp)
                nc.vector.tensor_tensor(
                    out=vv[:, :, 64:65], in0=vv[:, :, 64:65],
                    in1=msk_t[:, b0:b0 + gl].unsqueeze(2),
                    op=ALU.mult)
                nc.vector.tensor_tensor(
                    out=vv[:, :, 0:64], in0=gv[:, :, 0:64],
                    in1=vv[:, :, 64:65].broadcast_to([128, gl, 64]),
                    op=ALU.mult)
                for k in range(gl):
                    q = b0 + k
                    j = int(blk_of[q])
                    if q == firstq[j]:
                        psums2[j] = psa.tile([128, 130], F32, tag="ps",
                                             name=f"ps2_{j}")
                        nc.tensor.matmul(psums2[j][:, 0:65], lhsT=ident[:],
                                         rhs=v2self[:, j * 65:(j + 1) * 65],
                                         start=True, stop=False)
                    nc.tensor.matmul(
                        psums2[j][:, 0:65], lhsT=ident[:],
                        rhs=vt[:, k * 65:(k + 1) * 65],
                        start=False, stop=(q == lastq[j]))
                    if q == lastq[j]:
                        epilogue2(j, psums2.pop(j))
            for j in range(nb):
                if K[j] == 0:
                    psum = psa.tile([128, 130], F32, tag="ps",
                                    name=f"ps2_{j}")
                    nc.tensor.matmul(psum[:, 0:65], lhsT=ident[:],
                                     rhs=v2self[:, j * 65:(j + 1) * 65],
                                     start=True, stop=True)
                    epilogue2(j, psum)

    nc.finalize()
    return nc


def _prep_inputs(x, W1, att_src1, att_dst1, b1, W2, att_src2, att_dst2, b2,
                 sch):
    vpad, pc = sch["vpad"], sch["pc"]
    W1r = W1.reshape(F_IN, HEADS, HID)
    w1_aug = np.zeros((F_IN, 132), np.float32)
    w1_aug[:, 0:HEADS * HID] = W1
    for h in range(HEADS):
        w1_aug[:, HEADS * HID + h] = W1r[:, h, :] @ att_dst1[h]
        w1_aug[:, HEADS * HID + 2 + h] = W1r[:, h, :] @ att_src1[h]
    w2_aug = np.zeros((HEADS * HID, 66), np.float32)
    w2_aug[:, 0:OUT] = W2
    w2_aug[:, OUT] = W2 @ att_src2[0]
    w2_aug[:, OUT + 1] = W2 @ att_dst2[0]
    b1_rep = np.broadcast_to(b1, (128, HEADS * HID)).copy()
    b2_rep = np.broadcast_to(b2, (128, OUT)).copy()
    asrc_rep = np.zeros((128, 128), np.float32)
    for h in range(HEADS):
        asrc_rep[:, h * HID:(h + 1) * HID] = att_src1[h]
    x_rho = np.zeros((vpad, F_IN), np.float32)
    x_rho[sch["row_of_node"]] = x
    in_maps = []
    for c in range(NCORES):
        in_maps.append({
            "xt": np.ascontiguousarray(
                x_rho[c * pc:(c + 1) * pc].T).astype(BF16),
            "idx": sch["idx_wrapped"][c],
            "msk": sch["mask_stream"][c],
            "w1aug": w1_aug.astype(BF16),
            "w2aug": w2_aug.astype(BF16),
            "b1rep": b1_rep,
            "b2rep": b2_rep,
            "asrcrep": asrc_rep.astype(BF16),
        })
    return in_maps


def kernel(x, edge_index, W1, att_src1, att_dst1, b1, W2, att_src2, att_dst2,
           b2):
    from concourse import bass_utils

    x = np.asarray(x, np.float32)
    W1 = np.asarray(W1, np.float32)
    W2 = np.asarray(W2, np.float32)
    att_src1 = np.asarray(att_src1, np.float32)
    att_dst1 = np.asarray(att_dst1, np.float32)
    att_src2 = np.asarray(att_src2, np.float32)
    att_dst2 = np.asarray(att_dst2, np.float32)
    b1 = np.asarray(b1, np.float32)
    b2 = np.asarray(b2, np.float32)
    n_nodes = x.shape[0]

    sch = _build_schedule(edge_index, n_nodes)
    key = (sch["vpad"], sch["nch"], sch["K"], sch["cuts"])
    if key not in _cache:
        _cache[key] = _build_program(sch["vpad"], sch["pc"], sch["nb"],
                                     sch["K"], sch["nch"], sch["bias"],
                                     sch["cuts"], sch["tbases"],
                                     sch["batches"])
    nc = _cache[key]
    in_maps = _prep_inputs(x, W1, att_src1, att_dst1, b1, W2, att_src2,
                           att_dst2, b2, sch)
    res = bass_utils.run_bass_kernel_spmd(nc, in_maps,
                                          core_ids=list(range(NCORES)),
                                          trace=TRACE)
    kernel.last_exec_ns = res.exec_time_ns
    kernel.last_mean_ns = res.mean_exec_time_ns
    kernel.last_res = res
    out_all = np.concatenate([res.results[c]["out"] for c in range(NCORES)], 0)
    return out_all[sch["row_of_node"][:n_nodes]]
